# revision 1
# baseline (speedup 1.0000x reference)
"""DirectedEdgeConv (gnn_message_passing) Trainium2 kernel, 8-core SPMD.

out[e] = leaky_relu(edge_attr[e] @ Wself^T + b
                    + T_in[src[e]] + T_out[dst[e]], 0.2)
where T_in  = scatter_mean(edge_attr, dst) @ Win^T + b   [node table]
      T_out = scatter_mean(edge_attr, src) @ Wout^T      [node table]

Sharding strategy (graph partitioning):
  Phase A (node-sharded): core c owns nodes [c*NPC, (c+1)*NPC). The host
  groups the edge rows by (owning core, 128-node block) of their
  dst (resp. src) endpoint; each core streams its groups contiguously
  and segment-sums them via one-hot matmuls accumulated in PSUM, then
  scales by 1/cnt and applies the (linear) weight at node level. The
  self bias is baked into T_in.
  Phase B: AllGather the per-core [NBP, 128] table slices -> full
  [8*NBP, 128] tables on every core.
  Phase C (edge-sharded): core c streams its contiguous 75k-edge shard,
  computes h_self via PE (transpose + matmul), gathers T_in[src] /
  T_out[dst] rows with dma_gather (int16 indices; edges are grouped by
  (src-half, dst-half) of the table so indices fit 15 bits), adds and
  applies leaky relu.
"""

import os
import sys

sys.path.insert(0, "/opt/trn_rl_repo")

VARIANT = os.environ.get("KVARIANT", "barrier")

import numpy as np

import concourse.bacc as bacc
import concourse.bass as bass
import concourse.mybir as mybir
import concourse.tile as tile
from concourse import library_config
from concourse.bass_utils import run_bass_kernel_spmd
from concourse.masks import make_identity

P = 128
D = 128
C = 8  # cores
HALF = 32768  # int16 index capacity per dma_gather base

F32 = mybir.dt.float32
I16 = mybir.dt.int16

SUP = P * 12  # edges per supertile (KC=12)
KC = 12


def _cfg_full():
    return dict(E=600000, N=50000)


def _derive(cfg):
    E, N = cfg["E"], cfg["N"]
    assert N % C == 0 and E % C == 0
    NPC = N // C
    NB = (NPC + P - 1) // P
    NBP = NB * P
    EPC = E // C
    return NPC, NB, NBP, EPC


def build_kernel(cfg, KA_dst, KA_src, NS):
    """NS: list of 4 supertile counts per (src_hi*2+dst_hi) group."""
    E, N = cfg["E"], cfg["N"]
    NPC, NB, NBP, EPC = _derive(cfg)
    TROWS = C * NBP
    NSUP = sum(NS)
    SIDX = SUP // 16  # int16 idx tile free dim

    nc = bacc.Bacc(None, target_bir_lowering=False, debug=False)

    # ---- I/O ----
    agat_d = nc.dram_tensor("agat_d", [NB, P, KA_dst * D], F32, kind="ExternalInput")
    va_d = nc.dram_tensor("va_d", [NB, P, KA_dst], F32, kind="ExternalInput")
    agat_s = nc.dram_tensor("agat_s", [NB, P, KA_src * D], F32, kind="ExternalInput")
    va_s = nc.dram_tensor("va_s", [NB, P, KA_src], F32, kind="ExternalInput")
    invc_d = nc.dram_tensor("invc_d", [P, NB], F32, kind="ExternalInput")
    invc_s = nc.dram_tensor("invc_s", [P, NB], F32, kind="ExternalInput")
    xshard = nc.dram_tensor("xshard", [NSUP * P, KC * D], F32, kind="ExternalInput")
    gidx_in = nc.dram_tensor("gidx_in", [NSUP, P, SIDX], I16, kind="ExternalInput")
    gidx_out = nc.dram_tensor("gidx_out", [NSUP, P, SIDX], I16, kind="ExternalInput")
    wself = nc.dram_tensor("wself", [D, D], F32, kind="ExternalInput")
    win = nc.dram_tensor("win", [D, D], F32, kind="ExternalInput")
    wout = nc.dram_tensor("wout", [D, D], F32, kind="ExternalInput")
    bbc = nc.dram_tensor("bbc", [P, D], F32, kind="ExternalInput")
    iota_in = nc.dram_tensor("iota", [P, P], F32, kind="ExternalInput")
    y = nc.dram_tensor("y", [NSUP * P, KC * D], F32, kind="ExternalOutput")
    tdum = (
        nc.dram_tensor("tdummy", [TROWS, D], F32, kind="ExternalInput")
        if VARIANT == "gather_ext" else None
    )

    with tile.TileContext(nc) as tc:
        with (
            tc.tile_pool(name="const", bufs=1) as cpool,
            tc.tile_pool(name="sbuf", bufs=3) as pool,
            tc.tile_pool(name="small", bufs=4) as spool,
            tc.tile_pool(name="psum", bufs=2, space="PSUM") as psum,
            tc.tile_pool(name="dram", bufs=1, space="DRAM") as dram,
        ):
            nc.gpsimd.load_library(library_config.mlp)
            # constants
            ident = cpool.tile([P, P], F32)
            make_identity(nc, ident[:])
            iota_t = cpool.tile([P, P], F32)
            nc.sync.dma_start(out=iota_t[:], in_=iota_in[:])
            wself_t = cpool.tile([D, D], F32)
            nc.sync.dma_start(out=wself_t[:], in_=wself[:])
            win_t = cpool.tile([D, D], F32)
            nc.sync.dma_start(out=win_t[:], in_=win[:])
            wout_t = cpool.tile([D, D], F32)
            nc.sync.dma_start(out=wout_t[:], in_=wout[:])
            bbc_t = cpool.tile([P, D], F32)
            nc.sync.dma_start(out=bbc_t[:], in_=bbc[:])
            invc_d_t = cpool.tile([P, NB], F32)
            nc.sync.dma_start(out=invc_d_t[:], in_=invc_d[:])
            invc_s_t = cpool.tile([P, NB], F32)
            nc.sync.dma_start(out=invc_s_t[:], in_=invc_s[:])

            # collective dram buffers
            cc_in_d = dram.tile([NBP, D], F32)
            cc_out_d = dram.tile([TROWS, D], F32)
            cc_in_s = dram.tile([NBP, D], F32)
            cc_out_s = dram.tile([TROWS, D], F32)

            # ---- Phase A ----
            def phase_a(agat, va, KA, invc_t, w_t, cc_in, add_bias):
                for b in range(NB):
                    valt = spool.tile([P, KA], F32, tag="aval")
                    nc.sync.dma_start(out=valt[:], in_=va[b])
                    gat = pool.tile([P, KA * D], F32, tag="agather")
                    nc.sync.dma_start(out=gat[:], in_=agat[b])
                    ps = psum.tile([P, D], F32, tag="pA")
                    for j in range(KA):
                        oh = spool.tile([P, P], F32, tag="oh")
                        nc.vector.tensor_scalar(
                            oh[:], iota_t[:], valt[:, j : j + 1], None,
                            mybir.AluOpType.is_equal,
                        )
                        nc.tensor.matmul(
                            ps[:], oh[:], gat[:, j * D : (j + 1) * D],
                            start=(j == 0), stop=(j == KA - 1),
                        )
                    means = spool.tile([P, D], F32, tag="means")
                    nc.vector.tensor_scalar(
                        means[:], ps[:], invc_t[:, b : b + 1], None,
                        mybir.AluOpType.mult,
                    )
                    pst = psum.tile([P, D], F32, tag="pB")
                    nc.tensor.transpose(pst[:], means[:], ident[:])
                    meansT = spool.tile([P, D], F32, tag="meansT")
                    nc.scalar.copy(out=meansT[:], in_=pst[:])
                    psT = psum.tile([P, D], F32, tag="pC")
                    nc.tensor.matmul(psT[:], meansT[:], w_t[:], start=True, stop=True)
                    tt = spool.tile([P, D], F32, tag="tt")
                    if add_bias:
                        nc.vector.tensor_add(tt[:], psT[:], bbc_t[:])
                    else:
                        nc.scalar.copy(out=tt[:], in_=psT[:])
                    nc.sync.dma_start(out=cc_in[b * P : (b + 1) * P, :], in_=tt[:])

            phase_a(agat_d, va_d, KA_dst, invc_d_t, win_t, cc_in_d, True)
            nc.gpsimd.collective_compute(
                "AllGather", mybir.AluOpType.bypass,
                replica_groups=[list(range(C))],
                ins=[cc_in_d.opt()], outs=[cc_out_d.opt()],
            )
            phase_a(agat_s, va_s, KA_src, invc_s_t, wout_t, cc_in_s, False)
            nc.gpsimd.collective_compute(
                "AllGather", mybir.AluOpType.bypass,
                replica_groups=[list(range(C))],
                ins=[cc_in_s.opt()], outs=[cc_out_s.opt()],
            )

            if VARIANT != "nobarrier":
                tc.strict_bb_all_engine_barrier()

            # ---- Phase C ----
            def tbl_slice(cc_out, hi):
                if VARIANT == "gather_ext":
                    cc_out = tdum
                base = hi * HALF
                size = min(HALF, TROWS - base)
                return cc_out[base : base + size, :]

            s_global = 0
            for g in range(4):
                src_hi, dst_hi = g >> 1, g & 1
                for _ in range(NS[g]):
                    s = s_global
                    s_global += 1
                    sidx = spool.tile([P, SIDX], I16, tag="sidx")
                    nc.sync.dma_start(out=sidx[:], in_=gidx_in[s])
                    didx = spool.tile([P, SIDX], I16, tag="didx")
                    nc.sync.dma_start(out=didx[:], in_=gidx_out[s])
                    xt = pool.tile([P, KC * D], F32, tag="xt")
                    nc.sync.dma_start(out=xt[:], in_=xshard[s * P : (s + 1) * P, :])
                    gi = pool.tile([P, KC * D], F32, tag="gi")
                    go = pool.tile([P, KC * D], F32, tag="go")
                    if VARIANT == "nogather":
                        nc.vector.memset(gi[:], 0.0)
                        nc.vector.memset(go[:], 0.0)
                    else:
                        nc.gpsimd.dma_gather(
                            out_ap=gi[:].rearrange("p (j d) -> p j d", j=KC),
                            in_ap=tbl_slice(cc_out_d, src_hi),
                            idxs_ap=sidx[:],
                            num_idxs=SUP, num_idxs_reg=SUP, elem_size=D,
                            single_packet=False,
                        )
                        nc.gpsimd.dma_gather(
                            out_ap=go[:].rearrange("p (j d) -> p j d", j=KC),
                            in_ap=tbl_slice(cc_out_s, dst_hi),
                            idxs_ap=didx[:],
                            num_idxs=SUP, num_idxs_reg=SUP, elem_size=D,
                            single_packet=False,
                        )
                    yo = pool.tile([P, KC * D], F32, tag="yo")
                    for j in range(KC):
                        sl = slice(j * D, (j + 1) * D)
                        psx = psum.tile([P, D], F32, tag="pA")
                        nc.tensor.transpose(psx[:], xt[:, sl], ident[:])
                        xT = spool.tile([P, D], F32, tag="xT")
                        nc.scalar.copy(out=xT[:], in_=psx[:])
                        psh = psum.tile([P, D], F32, tag="pB")
                        nc.tensor.matmul(
                            psh[:], xT[:], wself_t[:], start=True, stop=True
                        )
                        s1 = spool.tile([P, D], F32, tag="s1")
                        nc.vector.tensor_add(s1[:], gi[:, sl], go[:, sl])
                        s2 = spool.tile([P, D], F32, tag="s2")
                        nc.vector.tensor_add(s2[:], psh[:], s1[:])
                        t1 = spool.tile([P, D], F32, tag="t1")
                        nc.scalar.mul(out=t1[:], in_=s2[:], mul=0.2)
                        nc.vector.tensor_max(yo[:, sl], s2[:], t1[:])
                    nc.sync.dma_start(out=y[s * P : (s + 1) * P, :], in_=yo[:])

    nc.compile()
    return nc


def prepare_inputs(cfg, edge_attr, edge_index, W_self_w, W_self_b, W_in_w, W_out_w):
    """Host-side sharding / graph partitioning. Returns (params, in_maps, post)."""
    E, N = cfg["E"], cfg["N"]
    NPC, NB, NBP, EPC = _derive(cfg)
    TROWS = C * NBP

    edge_attr = np.ascontiguousarray(edge_attr, dtype=np.float32)
    src = np.asarray(edge_index[0], dtype=np.int64)
    dst = np.asarray(edge_index[1], dtype=np.int64)

    wself = np.ascontiguousarray(np.asarray(W_self_w, np.float32).T)
    win = np.ascontiguousarray(np.asarray(W_in_w, np.float32).T)
    wout = np.ascontiguousarray(np.asarray(W_out_w, np.float32).T)
    bbc = np.tile(np.asarray(W_self_b, dtype=np.float32)[None, :], (P, 1))
    iota = np.tile(np.arange(P, dtype=np.float32)[None, :], (P, 1))

    # ---- phase A: group edge rows by (core, block) of endpoint ----
    def build_a(node_of_edge):
        core = node_of_edge // NPC
        local = node_of_edge - core * NPC
        blk = local >> 7
        inblk = (local & 127).astype(np.float32)
        key = (core * NB + blk).astype(np.int64)
        order = np.argsort(key, kind="stable")
        cnts = np.bincount(key, minlength=C * NB)
        KA = max(1, int(np.ceil(cnts.max() / P)))
        starts = np.zeros(C * NB, dtype=np.int64)
        np.cumsum(cnts[:-1], out=starts[1:])
        pos = np.arange(E, dtype=np.int64) - starts[key[order]]
        slot = key[order] * (P * KA) + pos  # flat (group, p*KA+j)
        agat = np.zeros((C * NB * P * KA, D), dtype=np.float32)
        agat[slot] = edge_attr[order]
        agat = agat.reshape(C, NB, P, KA * D)
        va = np.full((C * NB * P * KA), -1.0, dtype=np.float32)
        va[slot] = inblk[order]
        va = va.reshape(C, NB, P, KA)
        cnt_node = np.bincount(node_of_edge, minlength=N).astype(np.float32)
        inv = 1.0 / np.maximum(cnt_node, 1.0)
        inv_pad = np.zeros((C, NBP), dtype=np.float32)
        inv_pad[:, :NPC] = inv.reshape(C, NPC)
        invc = np.ascontiguousarray(inv_pad.reshape(C, NB, P).transpose(0, 2, 1))
        return KA, agat, va, invc

    KA_dst, agat_d, va_d, invc_d = build_a(dst)
    KA_src, agat_s, va_s, invc_s = build_a(src)

    # ---- phase C: 4-way (src_hi, dst_hi) grouping per core ----
    trow = lambda n: (n // NPC) * NBP + (n % NPC)
    src_rows = trow(src)
    dst_rows = trow(dst)
    grp = (src_rows >= HALF) * 2 + (dst_rows >= HALF)

    percore = []
    for c in range(C):
        lo, hi = c * EPC, (c + 1) * EPC
        g = grp[lo:hi]
        order = np.argsort(g, kind="stable")  # local edge order, grouped
        gcnt = np.bincount(g, minlength=4)
        percore.append((order, gcnt))
    NS = [
        max(1 if max(pc[1][g] for pc in percore) > 0 else 0,
            int(np.ceil(max(pc[1][g] for pc in percore) / SUP)))
        for g in range(4)
    ]
    NSUP = sum(NS)

    def wrap_idx(vals):
        # vals: [SUP] int -> [P, SIDX] int16 (16-partition wrap, replicated)
        S = SUP // 16
        t = np.zeros((16, S), dtype=np.int16)
        t[np.arange(SUP) % 16, np.arange(SUP) // 16] = vals.astype(np.int16)
        return np.tile(t, (8, 1))

    in_maps = []
    inv_perms = []
    for c in range(C):
        lo = c * EPC
        order, gcnt = percore[c]
        # slot list: for each group g, its edges then pad to NS[g]*SUP slots
        slot_edge = np.full(NSUP * SUP, -1, dtype=np.int64)  # local edge id or -1
        off = 0
        gstart = np.zeros(5, dtype=np.int64)
        np.cumsum(gcnt, out=gstart[1:])
        for g in range(4):
            cnt = gcnt[g]
            slot_edge[off : off + cnt] = order[gstart[g] : gstart[g] + cnt]
            off += NS[g] * SUP
        valid = slot_edge >= 0
        le = np.where(valid, slot_edge, 0)
        ge = le + lo  # global edge id (pad -> lo, masked later)
        # xshard: supertile s, slot gpos=j*128+p -> [s*P+p, j*D:(j+1)*D]
        xs = np.where(valid[:, None], edge_attr[ge], 0).astype(np.float32)
        xs = xs.reshape(NSUP, KC, P, D).transpose(0, 2, 1, 3).reshape(NSUP * P, KC * D)
        # gather indices (rebased per group)
        si = src_rows[ge].astype(np.int64)
        di = dst_rows[ge].astype(np.int64)
        off = 0
        for g in range(4):
            sl = slice(off, off + NS[g] * SUP)
            si[sl] -= (g >> 1) * HALF
            di[sl] -= (g & 1) * HALF
            off += NS[g] * SUP
        si = np.where(valid, si, 0)
        di = np.where(valid, di, 0)
        gin = np.stack([wrap_idx(si[s * SUP : (s + 1) * SUP]) for s in range(NSUP)])
        gout = np.stack([wrap_idx(di[s * SUP : (s + 1) * SUP]) for s in range(NSUP)])
        in_maps.append(
            dict(
                agat_d=agat_d[c], va_d=va_d[c], agat_s=agat_s[c], va_s=va_s[c],
                invc_d=invc_d[c], invc_s=invc_s[c],
                xshard=xs, gidx_in=gin, gidx_out=gout,
                wself=wself, win=win, wout=wout, bbc=bbc, iota=iota,
            )
        )
        inv_perms.append((slot_edge, valid))

    def postprocess(results):
        full = np.empty((E, D), dtype=np.float32)
        for c in range(C):
            yv = results[c]["y"].reshape(NSUP, P, KC, D).transpose(0, 2, 1, 3)
            yv = yv.reshape(NSUP * SUP, D)
            slot_edge, valid = inv_perms[c]
            full[c * EPC + slot_edge[valid]] = yv[valid]
        return full

    params = (KA_dst, KA_src, tuple(NS))
    return params, in_maps, postprocess


_NC_CACHE = {}


def run(cfg, inputs, trace=False, trace_kwargs=None):
    params, in_maps, post = prepare_inputs(
        cfg,
        inputs["edge_attr"],
        inputs["edge_index"],
        inputs["W_self_w"],
        inputs["W_self_b"],
        inputs["W_in_w"],
        inputs["W_out_w"],
    )
    key = (tuple(sorted(cfg.items())), params)
    if key not in _NC_CACHE:
        _NC_CACHE[key] = build_kernel(cfg, params[0], params[1], list(params[2]))
    nc = _NC_CACHE[key]
    kw = {}
    if trace:
        kw["trace"] = True
        if trace_kwargs:
            kw.update(trace_kwargs)
    res = run_bass_kernel_spmd(nc, in_maps, core_ids=list(range(C)), **kw)
    return post(res.results), res


def kernel(**inputs) -> np.ndarray:
    out, _ = run(_cfg_full(), inputs)
    return out.astype(np.float32)



# revision 16
# speedup vs baseline: 1.7280x; 1.7280x over previous
"""DirectedEdgeConv (gnn_message_passing) Trainium2 kernel, 8-core SPMD, bf16.

out[e] = leaky_relu(edge_attr[e] @ Wself^T + b
                    + T_in[src[e]] + T_out[dst[e]], 0.2)
where T_in  = scatter_mean(edge_attr, dst) @ Win^T + b   [node table]
      T_out = scatter_mean(edge_attr, src) @ Wout^T      [node table]

Design (v2, bf16 everywhere):
  Node partition: core c owns nodes [c*NPC, (c+1)*NPC).
  A2 (src-partitioned, src-sorted, block-uniform TB2 tiles/block): one-hot
    PE scatter (inv-count folded into the one-hot) -> own T_out slice ->
    DRAM -> AllGather (bf16) -> full table.
  A1 (dst-partitioned, dst-sorted, TB1): same scatter -> own T_in slice,
    kept in SBUF (bias baked in).  Runs while the AllGather is in flight.
  C (same edge order as A2, transposed stream [d, e]):
    z^T = Wself @ x^T  (PE, no per-tile transposes)
        + T_in[src]    (PE one-hot expansion from SBUF slice)
        + T_out[dst]   (dma_gather transpose=True from full table,
                        4 SWDGE queues round-robin; idx is int16 rebased
                        by -32768 so the full 50k-row table fits)
    LReLU via scalar mul + vector max; y written transposed bf16,
    host restores order/dtype.
"""

import os
import sys

sys.path.insert(0, "/opt/trn_rl_repo")

import numpy as np

import concourse.bacc as bacc
import concourse.bass as bass
import concourse.mybir as mybir
import concourse.tile as tile
from concourse import library_config
from concourse.bass_utils import run_bass_kernel_spmd

P = 128
D = 128
C = 8
HALF = 32768
SUP = 2048           # phase-C supertile (one gather per supertile)
CH = 512             # psum chunk width

F32 = mybir.dt.float32
BF16 = mybir.dt.bfloat16
I16 = mybir.dt.int16


def _cfg_full():
    return dict(E=600000, N=50000)


def _derive(cfg):
    N = cfg["N"]
    NPC = N // C
    NB = (NPC + P - 1) // P
    NBP = NB * P
    return NPC, NB, NBP


def build_kernel(cfg, TB1, TB2, TLO, THI, NSUP_LO, NSUP_HI):
    NPC, NB, NBP = _derive(cfg)
    TROWS = C * NBP
    NT1 = NB * TB1
    NT2 = NB * TB2
    NSUP = NSUP_LO + NSUP_HI
    ESUP = NSUP * SUP
    NTC = ESUP // P

    def tile_block(t):
        # static (core-independent) tile -> src-block map of the C stream
        if t < NSUP_LO * (SUP // P):
            b = t // TLO
        else:
            b = (t - NSUP_LO * (SUP // P)) // THI
        return min(b, NB - 1)  # pad tiles at group end -> clamp (va=-1 anyway)

    nc = bacc.Bacc(None, target_bir_lowering=False, debug=False,
                   num_swdge_queues=4)

    # ---- I/O ----
    xa1 = nc.dram_tensor("xa1", [NB, P, TB1 * D], BF16, kind="ExternalInput")
    va1 = nc.dram_tensor("va1", [P, NT1], F32, kind="ExternalInput")
    iv1 = nc.dram_tensor("iv1", [P, NT1], F32, kind="ExternalInput")
    xa2 = nc.dram_tensor("xa2", [NB, P, TB2 * D], BF16, kind="ExternalInput")
    va2 = nc.dram_tensor("va2", [P, NT2], F32, kind="ExternalInput")
    iv2 = nc.dram_tensor("iv2", [P, NT2], F32, kind="ExternalInput")
    xc = nc.dram_tensor("xc", [P, ESUP], BF16, kind="ExternalInput")
    ohc = nc.dram_tensor("ohc", [P, ESUP], BF16, kind="ExternalInput")
    gidx = nc.dram_tensor("gidx", [P, NSUP * (SUP // 16)], I16, kind="ExternalInput")
    wselfT = nc.dram_tensor("wselfT", [D, D], BF16, kind="ExternalInput")
    winT = nc.dram_tensor("winT", [D, D], BF16, kind="ExternalInput")
    woutT = nc.dram_tensor("woutT", [D, D], BF16, kind="ExternalInput")
    identb = nc.dram_tensor("identb", [D, D], BF16, kind="ExternalInput")
    iota_in = nc.dram_tensor("iota", [P, P], BF16, kind="ExternalInput")
    biascol = nc.dram_tensor("biascol", [P, 1], F32, kind="ExternalInput")
    y = nc.dram_tensor("y", [P, ESUP], BF16, kind="ExternalOutput")
    y2 = nc.dram_tensor("y2", [P, 3 * SUP], BF16, kind="ExternalOutput")

    with tile.TileContext(nc) as tc:
        with (
            tc.tile_pool(name="const", bufs=1) as cpool,
            tc.tile_pool(name="sbuf", bufs=3) as pool,
            tc.tile_pool(name="small", bufs=4) as spool,
            tc.tile_pool(name="dram", bufs=1, space="DRAM") as dram,
        ):
            nc.gpsimd.load_library(library_config.mlp)
            iota_t = cpool.tile([P, P], BF16)
            nc.sync.dma_start(out=iota_t[:], in_=iota_in[:])
            wselfT_t = cpool.tile([D, D], BF16)
            nc.sync.dma_start(out=wselfT_t[:], in_=wselfT[:])
            winT_t = cpool.tile([D, D], BF16)
            nc.sync.dma_start(out=winT_t[:], in_=winT[:])
            woutT_t = cpool.tile([D, D], BF16)
            nc.sync.dma_start(out=woutT_t[:], in_=woutT[:])
            identb_t = cpool.tile([D, D], BF16)
            nc.sync.dma_start(out=identb_t[:], in_=identb[:])
            biascol_t = cpool.tile([P, 1], F32)
            nc.sync.dma_start(out=biascol_t[:], in_=biascol[:])
            va1_t = cpool.tile([P, NT1], F32)
            nc.sync.dma_start(out=va1_t[:], in_=va1[:])
            iv1_t = cpool.tile([P, NT1], F32)
            nc.sync.dma_start(out=iv1_t[:], in_=iv1[:])
            va2_t = cpool.tile([P, NT2], F32)
            nc.sync.dma_start(out=va2_t[:], in_=va2[:])
            iv2_t = cpool.tile([P, NT2], F32)
            nc.sync.dma_start(out=iv2_t[:], in_=iv2[:])
            gidx_t = cpool.tile([P, NSUP * (SUP // 16)], I16)
            nc.sync.dma_start(out=gidx_t[:], in_=gidx[:])

            tin_sb = cpool.tile([P, NB * D], BF16)  # T_in slice [n_in_blk, b*D+d]

            cc_in = dram.tile([NBP, D], BF16)
            cc_out = dram.tile([TROWS, D], BF16)

            # ---- phase A (shared): block-uniform one-hot scatter ----
            def phase_a(psum, xa, va_t, iv_t, TB, w_t, out_sb, out_dram, add_bias):
                for b in range(NB):
                    xt = pool.tile([P, TB * D], BF16, tag="xat")
                    nc.sync.dma_start(out=xt[:], in_=xa[b])
                    sp = psum.tile([P, P], F32, tag="SA")
                    for j in range(TB):
                        t = b * TB + j
                        oh = spool.tile([P, P], BF16, tag="oh")
                        nc.vector.tensor_scalar(
                            oh[:], iota_t[:], va_t[:, t : t + 1],
                            iv_t[:, t : t + 1],
                            mybir.AluOpType.is_equal, mybir.AluOpType.mult,
                        )
                        nc.tensor.matmul(
                            sp[:], xt[:, j * D : (j + 1) * D], oh[:],
                            start=(j == 0), stop=(j == TB - 1),
                        )
                    # sp = S^T [d, n]
                    s_sb = spool.tile([P, P], BF16, tag="ssb")
                    nc.scalar.copy(out=s_sb[:], in_=sp[:])
                    tw = psum.tile([P, P], F32, tag="TW")
                    nc.tensor.matmul(tw[:], w_t[:], s_sb[:], start=True, stop=True)
                    # tw = T^T [d', n]
                    t2 = spool.tile([P, P], BF16, tag="t2")
                    if add_bias:
                        nc.vector.tensor_scalar(
                            t2[:], tw[:], biascol_t[:, 0:1], None,
                            mybir.AluOpType.add,
                        )
                    else:
                        nc.scalar.copy(out=t2[:], in_=tw[:])
                    tr = psum.tile([P, P], BF16, tag="TR")
                    nc.tensor.transpose(tr[:], t2[:], identb_t[:])
                    # tr = T [n, d']
                    if out_sb is not None:
                        nc.scalar.copy(out=out_sb[:, b * D : (b + 1) * D], in_=tr[:])
                    else:
                        t3 = spool.tile([P, P], BF16, tag="t3")
                        nc.scalar.copy(out=t3[:], in_=tr[:])
                        nc.sync.dma_start(
                            out=out_dram[b * P : (b + 1) * P, :], in_=t3[:]
                        )

            # A2 first: its AllGather overlaps A1
            with tc.tile_pool(name="psumA", bufs=2, space="PSUM") as psumA:
                phase_a(psumA, xa2, va2_t, iv2_t, TB2, woutT_t, None, cc_in, False)
                nc.gpsimd.collective_compute(
                    "AllGather", mybir.AluOpType.bypass,
                    replica_groups=[list(range(C))],
                    ins=[cc_in.opt()], outs=[cc_out.opt()],
                )
                phase_a(psumA, xa1, va1_t, iv1_t, TB1, winT_t, tin_sb, None, True)

            tc.strict_bb_all_engine_barrier()

            # ---- phase C ----
            psum_cm = tc.tile_pool(name="psumC", bufs=2, space="PSUM")
            psum = psum_cm.__enter__()
            tbl_lo = cc_out[0:HALF, :]
            tbl_hi = cc_out[HALF:TROWS, :]
            SIDX = SUP // 16
            # first 3 supertiles re-emitted at the end: their gathers can
            # race the AllGather landing right after the barrier
            for si, s in enumerate(list(range(NSUP)) + [0, 1, 2]):
                xcs = pool.tile([P, SUP], BF16, tag="xcs")
                nc.sync.dma_start(out=xcs[:], in_=xc[:, s * SUP : (s + 1) * SUP])
                ohst = pool.tile([P, SUP], BF16, tag="ohst")
                nc.sync.dma_start(out=ohst[:], in_=ohc[:, s * SUP : (s + 1) * SUP])
                go = pool.tile([P, SUP], BF16, tag="go")
                nc.gpsimd.dma_gather(
                    out_ap=go[:].rearrange("p (j d) -> p j d", j=SUP // P),
                    in_ap=(tbl_lo if s < NSUP_LO else tbl_hi),
                    idxs_ap=gidx_t[:, s * SIDX : (s + 1) * SIDX],
                    num_idxs=SUP, num_idxs_reg=SUP, elem_size=D,
                    transpose=False, single_packet=False, queue_num=si % 4,
                )
                ysup = pool.tile([P, SUP], BF16, tag="ysup")
                zsb = pool.tile([P, SUP], BF16, tag="zsb")
                for j in range(SUP // P):
                    tg = s * (SUP // P) + j
                    b = tile_block(tg)
                    off = j * P
                    zp = psum.tile([P, P], F32, tag="Z")
                    # z_nat[e, d] = x^T_tile.T @ WselfT  (+ Tin expand + Tout)
                    nc.tensor.matmul(
                        zp[:], xcs[:, off : off + P], wselfT_t[:],
                        start=True, stop=False,
                    )
                    nc.tensor.matmul(
                        zp[:], ohst[:, off : off + P],
                        tin_sb[:, b * D : (b + 1) * D],
                        start=False, stop=False,
                    )
                    nc.tensor.matmul(
                        zp[:], identb_t[:], go[:, off : off + P],
                        start=False, stop=True,
                    )
                    nc.scalar.copy(out=zsb[:, off : off + P], in_=zp[:])
                t1 = spool.tile([P, SUP], BF16, tag="t1")
                nc.scalar.mul(out=t1[:], in_=zsb[:], mul=0.2)
                nc.vector.tensor_max(ysup[:], zsb[:], t1[:])
                if si < NSUP:
                    nc.sync.dma_start(
                        out=y[:, s * SUP : (s + 1) * SUP], in_=ysup[:]
                    )
                else:
                    nc.sync.dma_start(
                        out=y2[:, s * SUP : (s + 1) * SUP], in_=ysup[:]
                    )
            psum_cm.__exit__(None, None, None)

    nc.compile()
    return nc


def prepare_inputs(cfg, edge_attr, edge_index, W_self_w, W_self_b, W_in_w, W_out_w):
    import ml_dtypes
    bf16 = ml_dtypes.bfloat16
    E, N = cfg["E"], cfg["N"]
    NPC, NB, NBP = _derive(cfg)

    edge_attr = np.ascontiguousarray(edge_attr, dtype=np.float32)
    src = np.asarray(edge_index[0], dtype=np.int64)
    dst = np.asarray(edge_index[1], dtype=np.int64)

    wselfT = np.ascontiguousarray(np.asarray(W_self_w, np.float32).T).astype(bf16)
    winT = np.ascontiguousarray(np.asarray(W_in_w, np.float32).T).astype(bf16)
    woutT = np.ascontiguousarray(np.asarray(W_out_w, np.float32).T).astype(bf16)
    identb = np.eye(D, dtype=np.float32).astype(bf16)
    iota = np.tile(np.arange(P, dtype=np.float32)[None, :], (P, 1)).astype(bf16)
    biascol = np.asarray(W_self_b, np.float32).reshape(P, 1)

    ea_bf = edge_attr.astype(bf16)

    # per-node inverse counts (torch_scatter mean semantics: clamp >= 1)
    cnt_dst = np.bincount(dst, minlength=N).astype(np.float32)
    cnt_src = np.bincount(src, minlength=N).astype(np.float32)
    inv_dst = 1.0 / np.maximum(cnt_dst, 1.0)
    inv_src = 1.0 / np.maximum(cnt_src, 1.0)

    # ---- block-uniform A-stream builder ----
    def build_a(node_of_edge, inv_node):
        owner = node_of_edge // NPC
        local = node_of_edge - owner * NPC
        blk = local >> 7
        percore = []
        maxtb = 1
        for c in range(C):
            sel = np.nonzero(owner == c)[0]
            order = sel[np.argsort(local[sel], kind="stable")]
            bcnt = np.bincount(blk[order], minlength=NB)
            maxtb = max(maxtb, int(np.ceil(bcnt.max() / P)))
            percore.append((order, bcnt))
        TB = maxtb
        xs = np.zeros((C, NB, P, TB * D), dtype=bf16)
        va = np.full((C, P, NB * TB), -1.0, dtype=np.float32)
        iv = np.zeros((C, P, NB * TB), dtype=np.float32)
        orders = []
        for c in range(C):
            order, bcnt = percore[c]
            starts = np.zeros(NB, dtype=np.int64)
            np.cumsum(bcnt[:-1], out=starts[1:])
            pos = np.arange(len(order)) - starts[blk[order]]
            slot = blk[order] * (TB * P) + pos  # row within padded stream
            # xs[c, b, p, j*D:(j+1)*D] = edge at (b, j*128+p)
            x_pad = np.zeros((NB * TB * P, D), dtype=bf16)
            x_pad[slot] = ea_bf[order]
            xs[c] = (x_pad.reshape(NB, TB, P, D).transpose(0, 2, 1, 3)
                     .reshape(NB, P, TB * D))
            tilecol = slot // P
            prow = slot % P
            va[c, prow, tilecol] = (local[order] & 127).astype(np.float32)
            iv[c, prow, tilecol] = inv_node[node_of_edge[order]]
            orders.append(order)
        return TB, xs, va, iv, orders

    TB1, xs1, va1, iv1, _ = build_a(dst, inv_dst)
    TB2, xs2, va2, iv2, _ = build_a(src, inv_src)

    trow = lambda n: (n // NPC) * NBP + (n % NPC)
    dst_row = trow(dst)
    hi = (dst_row >= HALF).astype(np.int64)

    # ---- C stream: (half, src-block)-uniform layout ----
    src_owner = src // NPC
    src_local = src - src_owner * NPC
    src_blk = src_local >> 7
    # per (core, block, half) counts -> uniform TLO/THI
    TLO = THI = 1
    perc = []
    for c in range(C):
        sel = np.nonzero(src_owner == c)[0]
        order = sel[np.argsort(src_local[sel] * 2 + hi[sel], kind="stable")]
        blk_o = src_blk[order]
        hi_o = hi[order]
        cl = np.bincount(blk_o[hi_o == 0], minlength=NB)
        ch = np.bincount(blk_o[hi_o == 1], minlength=NB)
        TLO = max(TLO, int(np.ceil(cl.max() / P)))
        THI = max(THI, int(np.ceil(ch.max() / P)))
        perc.append((order, cl, ch))
    NSUP_LO = int(np.ceil(NB * TLO * P / SUP))
    NSUP_HI = int(np.ceil(NB * THI * P / SUP))
    NSUP = NSUP_LO + NSUP_HI
    ESUP = NSUP * SUP
    NTC = ESUP // P
    HIBASE = NSUP_LO * SUP  # stream row where the hi group starts

    xcs = np.zeros((C, P, ESUP), dtype=bf16)
    ohcs = np.zeros((C, P, ESUP), dtype=bf16)
    gidx = np.zeros((C, P, NSUP * (SUP // 16)), dtype=np.int16)
    slots_all = []
    for c in range(C):
        order, cl, ch = perc[c]
        blk_o = src_blk[order]
        hi_o = hi[order]
        slot = np.zeros(len(order), dtype=np.int64)
        for h, cnts, base, TBH in ((0, cl, 0, TLO), (1, ch, HIBASE, THI)):
            idxs = np.nonzero(hi_o == h)[0]
            b_of = blk_o[idxs]
            o2 = np.argsort(b_of, kind="stable")
            bb = b_of[o2]
            st = np.zeros(NB, dtype=np.int64)
            np.cumsum(cnts[:-1], out=st[1:])
            runidx = np.arange(len(idxs)) - st[bb]  # position within block
            ordpos = np.empty(len(idxs), dtype=np.int64)
            ordpos[o2] = runidx
            slot[idxs] = base + b_of * (TBH * P) + ordpos
        x_pad = np.zeros((ESUP, D), dtype=bf16)
        x_pad[slot] = ea_bf[order]
        xcs[c] = x_pad.T
        ohcs[c][(src_local[order] & 127), slot] = 1.0
        gv = np.zeros(ESUP, dtype=np.int64)
        gv[slot] = dst_row[order] - hi[order] * HALF
        gi = np.zeros((16, ESUP // 16), dtype=np.int16)
        gi[np.arange(ESUP) % 16, np.arange(ESUP) // 16] = gv.astype(np.int16)
        gidx[c] = np.tile(gi, (8, 1))
        slots_all.append((order, slot))

    in_maps = []
    for c in range(C):
        in_maps.append(dict(
            xa1=xs1[c], va1=va1[c], iv1=iv1[c],
            xa2=xs2[c], va2=va2[c], iv2=iv2[c],
            xc=xcs[c], ohc=ohcs[c], gidx=gidx[c],
            wselfT=wselfT, winT=winT, woutT=woutT,
            identb=identb, iota=iota, biascol=biascol,
        ))

    def postprocess(results):
        full = np.empty((E, D), dtype=np.float32)
        for c in range(C):
            yT = np.asarray(results[c]["y"], dtype=np.float32)  # [128, ESUP]
            yT[:, : 3 * SUP] = np.asarray(results[c]["y2"], dtype=np.float32)
            # natural per-tile layout: yT[p, t*128+d] = z[edge(t,p), d]
            ynat = (yT.reshape(P, yT.shape[1] // P, P).transpose(1, 0, 2)
                    .reshape(-1, P))
            order, slot = slots_all[c]
            full[order] = ynat[slot]
        return full

    return (TB1, TB2, TLO, THI, NSUP_LO, NSUP_HI), in_maps, postprocess


_NC_CACHE = {}


def run(cfg, inputs, trace=False, trace_kwargs=None):
    params, in_maps, post = prepare_inputs(
        cfg,
        inputs["edge_attr"],
        inputs["edge_index"],
        inputs["W_self_w"],
        inputs["W_self_b"],
        inputs["W_in_w"],
        inputs["W_out_w"],
    )
    key = (tuple(sorted(cfg.items())), params)
    if key not in _NC_CACHE:
        _NC_CACHE[key] = build_kernel(cfg, *params)
    nc = _NC_CACHE[key]
    kw = {}
    if trace:
        kw["trace"] = True
        if trace_kwargs:
            kw.update(trace_kwargs)
    res = run_bass_kernel_spmd(nc, in_maps, core_ids=list(range(C)), **kw)
    return post(res.results), res


def kernel(**inputs) -> np.ndarray:
    out, _ = run(_cfg_full(), inputs)
    return out.astype(np.float32)
